# revision 1
# baseline (speedup 1.0000x reference)
"""Trainium2 Bass kernel for nn_AttentionMap (dense_transformer).

Computes, per (batch, head):
    dots = clip(q)@clip(k).T * SCALE + clip(pq)@clip(pk).T * REL_SCALE
    dots = where(mask, -inf, dots)
    out  = softmax(dots, axis=-1)

Sharding: the 32 (batch*head) pairs are split 4-per-core across 8
NeuronCores; each core computes its own [S, S] maps independently.

Device-side algorithm (per core, per head):
  - Inputs are staged host-side in a transposed, concatenated layout:
      qt[128, S]: rows 0:64 = q^T, 64:96 = pq^T, row 96 = 1.0, rest 0
      kt[128, S]: rows 0:64 = k^T, 64:96 = pk^T, row 96 = mask_bias, rest 0
    so ONE matmul with contraction dim 97 produces
    q@k^T*SCALE + pq@pk^T*REL + mask_bias (the scales are applied on-device
    to the q rows; the ones-row of qt picks up the mask bias from kt).
  - clip to [-5, 5] and scaling run on the vector engine on-device.
  - matmul in float32r (full PE throughput for N=512 moving tiles).
  - softmax: ACT Exp with accum_out produces exp(dots) and the row sums in
    a single pass (no max-subtraction: logits for this distribution are
    bounded well below exp overflow; masked entries are -1e9 -> exp == 0
    exactly, matching the reference's -inf handling).
  - DVE reciprocal + per-partition tensor_scalar multiply normalizes.
"""

from contextlib import ExitStack

import numpy as np

import concourse.bass as bass
import concourse.tile as tile
from concourse import bacc, mybir
from concourse.bass_utils import run_bass_kernel_spmd

B, H, S, D, DP = 2, 16, 2048, 64, 32
N_CORES = 8
HPC = (B * H) // N_CORES  # heads per core = 4
SCALE = float(D) ** -0.5
REL_POS_SCALE = float(DP) ** -0.5
MASK_BIAS = -1.0e9
QBLK = 128  # queries per block (PSUM partition dim)
NBLK = 512  # keys per matmul (one PSUM bank of fp32)
N_QBLK = S // QBLK  # 16
N_KBLK = S // NBLK  # 4
PREP = 1024  # prep chunk (columns)

TRACE = False  # set True (e.g. from test.py) to collect the neuron profile
LAST_RESULT = None  # BassKernelResults of the most recent run

_NC_CACHE = None


def _build_nc() -> bass.Bass:
    nc = bacc.Bacc("TRN2", target_bir_lowering=False, debug=False)
    f32 = mybir.dt.float32
    f32r = mybir.dt.float32r
    Alu = mybir.AluOpType

    qt_d = nc.declare_dram_parameter("qt", [HPC, 128, S], f32, isOutput=False)
    kt_d = nc.declare_dram_parameter("kt", [HPC, 128, S], f32, isOutput=False)
    out_d = nc.declare_dram_parameter("out", [HPC, S, S], f32, isOutput=True)

    with ExitStack() as ctx:
        tc = ctx.enter_context(tile.TileContext(nc))
        qk_pool = ctx.enter_context(tc.tile_pool(name="qk", bufs=2))
        psum_pool = ctx.enter_context(tc.tile_pool(name="ps", bufs=2, space="PSUM"))
        exp_pool = ctx.enter_context(tc.tile_pool(name="expv", bufs=3))
        out_pool = ctx.enter_context(tc.tile_pool(name="outv", bufs=3))
        stat_pool = ctx.enter_context(tc.tile_pool(name="stat", bufs=8))

        # dummy activation so the Exp table DMA overlaps the first loads
        dummy = stat_pool.tile([1, 1], f32, tag="dum")
        nc.vector.memset(dummy[:], 0.0)
        nc.scalar.activation(dummy[:], dummy[:], mybir.ActivationFunctionType.Exp)

        for h in range(HPC):
            # staging (f32, straight from DRAM)
            qs = qk_pool.tile([128, S], f32, tag="qs")
            ks = qk_pool.tile([128, S], f32, tag="ks")
            # matmul operands must be written as float32r (fp32r matmuls
            # require producers to round); clip/scale DVE ops do the cast.
            qr = qk_pool.tile([97, S], f32r, tag="qr")
            kr = qk_pool.tile([97, S], f32r, tag="kr")

            # whole-head contiguous loads (column-sliced DRAM reads would be
            # 2KB-strided -> small-descriptor HBM penalty); SWDGE so they
            # don't head-of-line-block the SP sequencer issuing out-DMAs
            nc.gpsimd.dma_start(out=qs[:], in_=qt_d[h])
            nc.gpsimd.dma_start(out=ks[:], in_=kt_d[h])

            # prep in column chunks so the first matmuls aren't gated on
            # whole-row DVE ops
            for c in range(S // PREP):
                col = slice(c * PREP, (c + 1) * PREP)
                # keys/pos_keys: clip to [-5, 5] (row 96 = mask bias row:
                # copied verbatim)
                nc.vector.tensor_scalar(
                    out=kr[0:96, col], in0=ks[0:96, col],
                    scalar1=5.0, scalar2=-5.0, op0=Alu.min, op1=Alu.max,
                )
                nc.vector.tensor_copy(kr[96:97, col], ks[96:97, col])
                # queries: clip, then scale content rows by SCALE and pos
                # rows by REL_POS_SCALE; row 96 is the ones-row
                nc.vector.tensor_scalar(
                    out=qr[0:96, col], in0=qs[0:96, col],
                    scalar1=5.0, scalar2=-5.0, op0=Alu.min, op1=Alu.max,
                )
                nc.vector.tensor_scalar_mul(qr[0:64, col], qr[0:64, col], SCALE)
                nc.vector.tensor_scalar_mul(
                    qr[64:96, col], qr[64:96, col], REL_POS_SCALE
                )
                nc.vector.tensor_copy(qr[96:97, col], qs[96:97, col])

            for qb in range(N_QBLK):
                ps = psum_pool.tile([128, S], f32)
                for kb in range(N_KBLK):
                    nc.tensor.matmul(
                        ps[:, kb * NBLK:(kb + 1) * NBLK],
                        lhsT=qr[:, qb * QBLK:(qb + 1) * QBLK],
                        rhs=kr[:, kb * NBLK:(kb + 1) * NBLK],
                        start=True, stop=True,
                    )
                ev = exp_pool.tile([128, S], f32)
                sm = stat_pool.tile([128, 1], f32, tag="sm")
                nc.scalar.activation(
                    ev[:], ps[:], mybir.ActivationFunctionType.Exp,
                    accum_out=sm[:],
                )
                rc = stat_pool.tile([128, 1], f32, tag="rc")
                nc.vector.reciprocal(rc[:], sm[:])
                ov = out_pool.tile([128, S], f32)
                nc.vector.tensor_scalar_mul(ov[:], ev[:], rc[:])
                nc.sync.dma_start(
                    out=out_d[h, qb * QBLK:(qb + 1) * QBLK, :], in_=ov[:]
                )
    return nc


def _get_nc() -> bass.Bass:
    global _NC_CACHE
    if _NC_CACHE is None:
        nc = _build_nc()
        nc.finalize()
        _NC_CACHE = nc
    return _NC_CACHE


def kernel(keys, queries, pos_key, pos_query, mask) -> np.ndarray:
    global LAST_RESULT
    keys = np.ascontiguousarray(np.asarray(keys, dtype=np.float32))
    queries = np.ascontiguousarray(np.asarray(queries, dtype=np.float32))
    pos_key = np.ascontiguousarray(np.asarray(pos_key, dtype=np.float32))
    pos_query = np.ascontiguousarray(np.asarray(pos_query, dtype=np.float32))
    mask = np.asarray(mask)

    q = queries.reshape(B * H, S, D)
    k = keys.reshape(B * H, S, D)
    pq = pos_query.reshape(B * H, S, DP)
    pk = pos_key.reshape(B * H, S, DP)
    mask_bias = np.where(mask, np.float32(MASK_BIAS), np.float32(0.0))  # [B, S]

    in_maps = []
    for c in range(N_CORES):
        sel = slice(c * HPC, (c + 1) * HPC)
        qt = np.zeros((HPC, 128, S), np.float32)
        kt = np.zeros((HPC, 128, S), np.float32)
        qt[:, 0:D, :] = q[sel].transpose(0, 2, 1)
        qt[:, D:D + DP, :] = pq[sel].transpose(0, 2, 1)
        qt[:, D + DP, :] = 1.0
        kt[:, 0:D, :] = k[sel].transpose(0, 2, 1)
        kt[:, D:D + DP, :] = pk[sel].transpose(0, 2, 1)
        for i in range(HPC):
            b = (c * HPC + i) // H
            kt[i, D + DP, :] = mask_bias[b]
        in_maps.append({"qt": qt, "kt": kt})

    res = run_bass_kernel_spmd(
        _get_nc(), in_maps, core_ids=list(range(N_CORES)), trace=TRACE
    )
    LAST_RESULT = res
    out = np.concatenate([res.results[c]["out"] for c in range(N_CORES)], axis=0)
    return out.reshape(B, H, S, S)



# revision 2
# speedup vs baseline: 1.0411x; 1.0411x over previous
"""Trainium2 Bass kernel for nn_AttentionMap (dense_transformer).

Computes, per (batch, head):
    dots = clip(q)@clip(k).T * SCALE + clip(pq)@clip(pk).T * REL_SCALE
    dots = where(mask, -inf, dots)
    out  = softmax(dots, axis=-1)

Sharding: the 32 (batch*head) pairs are split 4-per-core across 8
NeuronCores; each core computes its own [S, S] maps independently.

Key optimizations over the v1 kernel (228.6 us, HBM-write bound):
  - Masked-key compaction. mask is shared by all heads of a batch and
    masked columns of the output are exactly 0 (softmax of -inf), so the
    host gathers only the unmasked key columns (~1024 of 2048), the
    device computes [S, U_pad] maps, and the host scatters them into the
    zero-initialized full output during unsharding. Halves matmul, exp,
    normalize AND the dominant output DMA traffic. Pad columns carry a
    -1e9 bias so exp underflows to exactly 0 (no effect on row sums).
  - bf16 output (rel err ~2^-9, far inside the 2e-2 gate): halves the
    remaining output-write bytes. Host converts back to f32.
  - Inputs are staged host-side in a transposed, concatenated layout:
      qt[97, S]:     rows 0:64 = q^T*SCALE, 64:96 = pq^T*REL, row 96 = 1.0
      kt[97, U_pad]: rows 0:64 = k^T, 64:96 = pk^T, row 96 = pad bias
    so ONE matmul with contraction dim 97 produces the full logits (the
    ones-row of qt picks up the bias row of kt). Scales are folded into
    q host-side; the clip (the module's nonlinearity) runs on-device
    with scale-adjusted bounds (clip(s*q, +-5s) == s*clip(q, +-5)).
  - DMA lands directly in the matmul operand tiles (f32 bytes into
    f32r tiles); clips run in place on DVE - no staging copies.
  - PSUM tiles padded to whole banks so TensorE writes and ScalarE
    reads never collide on a bank and fully overlap.
  - softmax: ACT Exp with accum_out produces exp(dots) in bf16 and the
    f32 row sums in a single pass (no max-subtraction: logits for this
    distribution are bounded well below exp overflow; masked/pad entries
    are -1e9 -> exp == 0 exactly, matching the reference's -inf).
  - DVE reciprocal + per-partition tensor_scalar bf16 multiply (2x/4x
    DVE mode) normalizes; sync-queue DMA streams bf16 tiles out.
"""

from contextlib import ExitStack

import numpy as np

import concourse.bass as bass
import concourse.tile as tile
from concourse import bacc, mybir
from concourse.bass_utils import run_bass_kernel_spmd

B, H, S, D, DP = 2, 16, 2048, 64, 32
N_CORES = 8
HPC = (B * H) // N_CORES  # heads per core = 4
SCALE = float(D) ** -0.5
REL_POS_SCALE = float(DP) ** -0.5
MASK_BIAS = -1.0e9
QBLK = 128  # queries per block (PSUM partition dim)
N_QBLK = S // QBLK  # 16
KROWS = D + DP + 1  # 97: contraction dim (content + pos + bias row)

TRACE = False  # set True (e.g. from test.py) to collect the neuron profile
LAST_RESULT = None  # BassKernelResults of the most recent run

_NC_CACHE = {}  # u_pad -> finalized Bass


def _build_nc(u_pad: int) -> bass.Bass:
    nc = bacc.Bacc("TRN2", target_bir_lowering=False, debug=False)
    f32 = mybir.dt.float32
    f32r = mybir.dt.float32r
    bf16 = mybir.dt.bfloat16
    Alu = mybir.AluOpType

    # psum tile padded to whole 512-f32 banks: a matmul output must stay
    # inside one bank, and bank-sharing between ring slots would serialize
    # TensorE writes against ScalarE reads (bank collisions are fatal).
    u_banks = -(-u_pad // 512) * 512

    qt_d = nc.declare_dram_parameter("qt", [HPC, KROWS, S], f32, isOutput=False)
    kt_d = nc.declare_dram_parameter("kt", [HPC, KROWS, u_pad], f32, isOutput=False)
    out_d = nc.declare_dram_parameter("out", [HPC, S, u_pad], bf16, isOutput=True)

    with ExitStack() as ctx:
        tc = ctx.enter_context(tile.TileContext(nc))
        qk_pool = ctx.enter_context(tc.tile_pool(name="qk", bufs=2))
        psum_pool = ctx.enter_context(tc.tile_pool(name="ps", bufs=2, space="PSUM"))
        exp_pool = ctx.enter_context(tc.tile_pool(name="expv", bufs=3))
        out_pool = ctx.enter_context(tc.tile_pool(name="outv", bufs=3))
        stat_pool = ctx.enter_context(tc.tile_pool(name="stat", bufs=8))

        # dummy activation so the Exp table DMA overlaps the first loads
        dummy = stat_pool.tile([1, 1], f32, tag="dum")
        nc.vector.memset(dummy[:], 0.0)
        nc.scalar.activation(dummy[:], dummy[:], mybir.ActivationFunctionType.Exp)

        for h in range(HPC):
            # DMA straight into the matmul operands (f32 bytes, f32r tag);
            # row 96 (ones / bias) needs no further processing. SWDGE so the
            # loads don't head-of-line-block the sync queue issuing out-DMAs.
            qr = qk_pool.tile([KROWS, S], f32r, tag="qr")
            kr = qk_pool.tile([KROWS, u_pad], f32r, tag="kr")
            nc.gpsimd.dma_start(out=qr[:], in_=qt_d[h])
            nc.gpsimd.dma_start(out=kr[:], in_=kt_d[h])

            # in-place clips; q rows were pre-scaled host-side, so the clip
            # bounds are scaled too (clip(s*q, +-5s) == s*clip(q, +-5)).
            nc.vector.tensor_scalar(
                out=qr[0:D, :], in0=qr[0:D, :],
                scalar1=5.0 * SCALE, scalar2=-5.0 * SCALE,
                op0=Alu.min, op1=Alu.max,
            )
            nc.vector.tensor_scalar(
                out=qr[D:D + DP, :], in0=qr[D:D + DP, :],
                scalar1=5.0 * REL_POS_SCALE, scalar2=-5.0 * REL_POS_SCALE,
                op0=Alu.min, op1=Alu.max,
            )
            nc.vector.tensor_scalar(
                out=kr[0:D + DP, :], in0=kr[0:D + DP, :],
                scalar1=5.0, scalar2=-5.0, op0=Alu.min, op1=Alu.max,
            )

            for qb in range(N_QBLK):
                ps = psum_pool.tile([128, u_pad], f32, padded_shape=[128, u_banks])
                for ofs in range(0, u_pad, 512):
                    n = min(512, u_pad - ofs)
                    nc.tensor.matmul(
                        ps[:, ofs:ofs + n],
                        lhsT=qr[:, qb * QBLK:(qb + 1) * QBLK],
                        rhs=kr[:, ofs:ofs + n],
                        start=True, stop=True,
                    )
                ev = exp_pool.tile([128, u_pad], bf16, tag="ev")
                sm = stat_pool.tile([128, 1], f32, tag="sm")
                nc.scalar.activation(
                    ev[:], ps[:], mybir.ActivationFunctionType.Exp,
                    accum_out=sm[:],
                )
                rc = stat_pool.tile([128, 1], f32, tag="rc")
                nc.vector.reciprocal(rc[:], sm[:])
                ov = out_pool.tile([128, u_pad], bf16, tag="ov")
                nc.vector.tensor_scalar_mul(ov[:], ev[:], rc[:])
                nc.sync.dma_start(
                    out=out_d[h, qb * QBLK:(qb + 1) * QBLK, :], in_=ov[:]
                )
    return nc


def _get_nc(u_pad: int) -> bass.Bass:
    if u_pad not in _NC_CACHE:
        nc = _build_nc(u_pad)
        nc.finalize()
        _NC_CACHE[u_pad] = nc
    return _NC_CACHE[u_pad]


def kernel(keys, queries, pos_key, pos_query, mask) -> np.ndarray:
    global LAST_RESULT
    keys = np.asarray(keys, dtype=np.float32)
    queries = np.asarray(queries, dtype=np.float32)
    pos_key = np.asarray(pos_key, dtype=np.float32)
    pos_query = np.asarray(pos_query, dtype=np.float32)
    mask = np.asarray(mask)

    q = queries.reshape(B * H, S, D)
    k = keys.reshape(B * H, S, D)
    pq = pos_query.reshape(B * H, S, DP)
    pk = pos_key.reshape(B * H, S, DP)

    # unmasked key columns per batch (masked columns are exactly 0 in the
    # softmax output and are filled host-side during unsharding)
    cols = [np.flatnonzero(~mask[b]) for b in range(B)]
    u_max = max(len(c) for c in cols)
    u_pad = min(S, max(512, -(-u_max // 128) * 128))

    in_maps = []
    for c in range(N_CORES):
        sel = slice(c * HPC, (c + 1) * HPC)
        b = (c * HPC) // H  # all heads of a core belong to one batch
        cb = cols[b]
        u = len(cb)
        qt = np.empty((HPC, KROWS, S), np.float32)
        qt[:, 0:D, :] = q[sel].transpose(0, 2, 1) * SCALE
        qt[:, D:D + DP, :] = pq[sel].transpose(0, 2, 1) * REL_POS_SCALE
        qt[:, D + DP, :] = 1.0
        kt = np.zeros((HPC, KROWS, u_pad), np.float32)
        kt[:, 0:D, :u] = k[sel][:, cb, :].transpose(0, 2, 1)
        kt[:, D:D + DP, :u] = pk[sel][:, cb, :].transpose(0, 2, 1)
        kt[:, D + DP, :u] = 0.0
        kt[:, D + DP, u:] = MASK_BIAS
        in_maps.append({"qt": qt, "kt": kt})

    res = run_bass_kernel_spmd(
        _get_nc(u_pad), in_maps, core_ids=list(range(N_CORES)), trace=TRACE
    )
    LAST_RESULT = res

    dev = np.stack(
        [np.asarray(res.results[c]["out"]) for c in range(N_CORES)], axis=0
    )  # [N_CORES, HPC, S, u_pad] bf16
    dev = dev.reshape(B, H, S, u_pad)
    full = np.zeros((B, H, S, S), np.float32)
    for b in range(B):
        cb = cols[b]
        full[b][:, :, cb] = dev[b][:, :, : len(cb)].astype(np.float32)
    return full
